# revision 1
# baseline (speedup 1.0000x reference)
"""Trainium2 Bass kernel for nn_Attention_LoRA_FFT.

Sharding: data-parallel over batch B=8 across the 8 NeuronCores. The DCT
LoRA weight reconstruction is sharded too: each core builds a 256-column
slice of one of WkT/WvT (chosen by per-core input data, the program is
identical) and an AllGather distributes the full weights.

Per-core device program:
  A) G = Sw.T @ Bmq ; Wpart = Bm.T @ G       (f32r, 1/8 of the work)
     AllGather -> full WkT, WvT
  B) qT = W_q @ x.T                          (f32r, [feat, tok], hides gather)
     kT = W_k @ x.T + WkT-apply              (f32r -> bf16 out)
     V' = [x @ W_v.T + x @ Wv.T | 1]         (f32r -> bf16, [tok, feat|1])
  C) per head pair: S.T = kT_h.T @ qT_h      (bf16 K=64, row-packed pairs)
     P.T = exp(S.T/8)  (ACT, psum->bf16; no max-subtraction: scores are O(10))
     [O.T ; Z] = V'.T @ P.T                  (bf16 M=65, ones col gives Z)
     O.T *= bcast(1/Z)  (GPSIMD partition_broadcast + DVE fast reciprocal)
     The V' build and the first output-projection half are interleaved into
     phase C to keep the PE fed (and the HAM clock-gate warm) while ACT
     chews through the exps.
  D) y.T = W_proj @ O.T + b                  (f32r) -> DMA out, host transposes
"""

import os
import sys

for _p in ("/opt/trn_rl_repo", "/root/.axon_site/_ro/trn_rl_repo"):
    if os.path.isdir(_p) and _p not in sys.path:
        sys.path.insert(0, _p)

import numpy as np

import concourse.bacc as bacc
import concourse.mybir as mybir
from concourse.tile import TileContext
from concourse.bass_utils import run_bass_kernel_spmd

B, N, C = 8, 1024, 1024
H, HD = 16, 64
NCORES = 8
PC = C // 128
F32 = mybir.dt.float32
F32R = mybir.dt.float32r
BF16 = mybir.dt.bfloat16
EXP = mybir.ActivationFunctionType.Exp


def _dct_matrix(n: int) -> np.ndarray:
    i = np.arange(n, dtype=np.float32)[:, None]
    j = np.arange(n, dtype=np.float32)[None, :]
    m = np.sqrt(np.float32(2.0 / n)) * np.cos(
        np.float32(np.pi) * i * (2.0 * j + 1.0) / np.float32(2.0 * n)
    )
    m[0, :] = np.sqrt(np.float32(1.0 / n))
    return m.astype(np.float32)


def _build():
    nc = bacc.Bacc("TRN2", target_bir_lowering=False, debug=False, num_devices=NCORES)

    xT_d = nc.dram_tensor("xT", [C, N], F32R, kind="ExternalInput")
    wqkvT_d = nc.dram_tensor("wqkvT", [C, 3 * C], F32R, kind="ExternalInput")
    wprojT_d = nc.dram_tensor("wprojT", [C, C], BF16, kind="ExternalInput")
    bias_d = nc.dram_tensor("bias", [C, 1], F32, kind="ExternalInput")
    bm_d = nc.dram_tensor("bm", [C, C], F32R, kind="ExternalInput")
    sw_d = nc.dram_tensor("sw", [C, C], F32R, kind="ExternalInput")
    bmq_d = nc.dram_tensor("bmq", [C, 256], F32R, kind="ExternalInput")
    yT_d = nc.dram_tensor("yT", [C, N], F32, kind="ExternalOutput")
    cc_in0 = nc.dram_tensor("cc_in0", [C, 128], F32R)
    cc_in1 = nc.dram_tensor("cc_in1", [C, 128], F32R)
    cc_out0 = nc.dram_tensor("cc_out0", [NCORES * C, 128], F32R, addr_space="Shared")
    cc_out1 = nc.dram_tensor("cc_out1", [NCORES * C, 128], F32R, addr_space="Shared")

    def chunked(dram_ap, sb):
        for cc in range(PC):
            nc.sync.dma_start(
                out=sb[:, cc, :], in_=dram_ap[cc * 128 : (cc + 1) * 128, :]
            )

    def col_slab(dram_ap, pool, tag, f0, width):
        slab = pool.tile([128, PC, width], F32R, tag=tag, name=tag)
        nc.sync.dma_start(
            out=slab[:],
            in_=dram_ap[:, f0 : f0 + width].rearrange("(cc p) f -> p cc f", p=128),
        )
        return slab

    with TileContext(nc) as tc:
        # ---------------- left stack ----------------
        small_p = tc.alloc_tile_pool(name="small", bufs=1, side="left")
        bias_sb = small_p.tile([128, PC, 1], F32, tag="bias")
        nc.sync.dma_start(
            out=bias_sb[:], in_=bias_d.rearrange("(cc p) o -> p cc o", p=128)
        )

        wv_p = tc.alloc_tile_pool(name="wvp", bufs=1, side="left")
        x_p = tc.alloc_tile_pool(name="xp", bufs=1, side="left")
        wk_p = tc.alloc_tile_pool(name="wkp", bufs=1, side="left")
        wv_sb = wv_p.tile([128, PC, C], F32R, tag="wv")
        wk_sb = wk_p.tile([128, PC, C], F32R, tag="wk")
        x_sb = x_p.tile([128, PC, N], F32R, tag="x")

        # ================= Phase A: sharded LoRA reconstruction =======
        slabA_p = tc.alloc_tile_pool(name="slabA", bufs=3, side="right")
        bm_p = tc.alloc_tile_pool(name="bmp", bufs=1, side="right")
        bmq_p = tc.alloc_tile_pool(name="bmqp", bufs=1, side="right")
        g_p = tc.alloc_tile_pool(name="gp", bufs=1, side="right")
        wpart_p = tc.alloc_tile_pool(name="wpartp", bufs=1, side="right")
        psA = tc.alloc_tile_pool(name="psA", bufs=4, space="PSUM")

        bm_sb = bm_p.tile([128, PC, C], F32R, tag="bm")
        chunked(bm_d, bm_sb)
        bmq_sb = bmq_p.tile([128, PC, 256], F32R, tag="bmq")
        nc.sync.dma_start(
            out=bmq_sb[:], in_=bmq_d.rearrange("(cc p) f -> p cc f", p=128)
        )

        g_sb = g_p.tile([128, PC, 256], F32R, tag="g", name="g_sb")
        wpart_sb = wpart_p.tile([128, PC, 256], F32R, tag="wpart", name="wpart_sb")
        for at in range(PC):
            slab = col_slab(sw_d, slabA_p, "slabA", at * 128, 128)
            ps = psA.tile([128, 256], F32, tag="psA", name="psA_t")
            for bc in range(PC):
                nc.tensor.matmul(
                    ps[:],
                    slab[:, bc, :],
                    bmq_sb[:, bc, :],
                    start=(bc == 0),
                    stop=(bc == PC - 1),
                )
            nc.scalar.copy(g_sb[:, at, :], ps[:])
        for ct in range(PC):
            ps = psA.tile([128, 256], F32, tag="psA", name="psA_t")
            for ac in range(PC):
                nc.tensor.matmul(
                    ps[:],
                    bm_sb[:, ac, ct * 128 : (ct + 1) * 128],
                    g_sb[:, ac, :],
                    start=(ac == 0),
                    stop=(ac == PC - 1),
                )
            nc.scalar.copy(wpart_sb[:, ct, :], ps[:])

        for hf, cc_in in ((0, cc_in0), (1, cc_in1)):
            nc.scalar.dma_start(
                out=cc_in.rearrange("(ct p) f -> p ct f", p=128),
                in_=wpart_sb[:, :, hf * 128 : (hf + 1) * 128],
            )

        psA.release()
        wpart_p.release()
        g_p.release()
        bmq_p.release()
        bm_p.release()
        slabA_p.release()

        # ================= Phase B =====================================
        chunked(xT_d, x_sb)
        kt_p = tc.alloc_tile_pool(name="ktp", bufs=1, side="right")
        qt_p = tc.alloc_tile_pool(name="qtp", bufs=1, side="right")
        vp_p = tc.alloc_tile_pool(name="vpp", bufs=1, side="right")
        slabB_p = tc.alloc_tile_pool(name="slabB", bufs=3, side="right")
        psB = tc.alloc_tile_pool(name="psB", bufs=2, space="PSUM", side="right")

        kT_sb = kt_p.tile([128, PC, N], BF16, tag="kT")
        qT_sb = qt_p.tile([128, PC, N], BF16, tag="qT")
        vp_sb = vp_p.tile([128, PC, H, HD + 1], BF16, tag="vp")

        # ---- qT (no lora dependency: covers the collective) ----
        for fc in range(PC):
            slab = col_slab(wqkvT_d, slabB_p, "slabB", fc * 128, 128)
            for th in range(2):
                ps = psB.tile([128, 512], F32, tag="psB", name="psB_t")
                for cc in range(PC):
                    nc.tensor.matmul(
                        ps[:],
                        slab[:, cc, :],
                        x_sb[:, cc, th * 512 : (th + 1) * 512],
                        start=(cc == 0),
                        stop=(cc == PC - 1),
                    )
                nc.scalar.copy(qT_sb[:, fc, th * 512 : (th + 1) * 512], ps[:])

        # AllGather the WkT/WvT quarters in two column-halves so the kT
        # section can start on even 128-col slices while the second half is
        # still on the wire; read back on the gpsimd DMA queue so the sync
        # queue keeps streaming weight slabs.
        for hf, cc_in, cc_out in ((0, cc_in0, cc_out0), (1, cc_in1, cc_out1)):
            nc.gpsimd.collective_compute(
                "AllGather",
                mybir.AluOpType.bypass,
                replica_groups=[list(range(NCORES))],
                ins=[cc_in[:]],
                outs=[cc_out[:]],
            )
            for wi, w_sb in ((0, wk_sb), (1, wv_sb)):
                for fq in range(4):
                    base = (wi * 4 + fq) * C
                    nc.gpsimd.dma_start(
                        out=w_sb[
                            :, :, fq * 256 + hf * 128 : fq * 256 + (hf + 1) * 128
                        ],
                        in_=cc_out[base : base + C, :].rearrange(
                            "(cc p) f -> p cc f", p=128
                        ),
                    )

        # ---- kT = qkv-k + lora-k (even fc first: needs only gather 0) ----
        for fc in [0, 2, 4, 6, 1, 3, 5, 7]:
            slab = col_slab(wqkvT_d, slabB_p, "slabB", C + fc * 128, 128)
            for th in range(2):
                ps = psB.tile([128, 512], F32, tag="psB", name="psB_t")
                for cc in range(PC):
                    nc.tensor.matmul(
                        ps[:],
                        slab[:, cc, :],
                        x_sb[:, cc, th * 512 : (th + 1) * 512],
                        start=(cc == 0),
                        stop=False,
                    )
                for cc in range(PC):
                    nc.tensor.matmul(
                        ps[:],
                        wk_sb[:, cc, fc * 128 : (fc + 1) * 128],
                        x_sb[:, cc, th * 512 : (th + 1) * 512],
                        start=False,
                        stop=(cc == PC - 1),
                    )
                nc.scalar.copy(kT_sb[:, fc, th * 512 : (th + 1) * 512], ps[:])
        slabB_p.release()
        wk_p.release()

        # ================= Phase C (+ interleaved V' and proj) =========
        ot_p = tc.alloc_tile_pool(name="otp", bufs=1, side="right")
        wps_p = tc.alloc_tile_pool(name="wpsp", bufs=2, side="right")
        y_p = tc.alloc_tile_pool(name="yp", bufs=2, side="right")
        pt_p = tc.alloc_tile_pool(name="ptp", bufs=2, side="right")
        rz_p = tc.alloc_tile_pool(name="rzp", bufs=2, side="right")
        zb_p = tc.alloc_tile_pool(name="zbp", bufs=1, side="right")
        psS = tc.alloc_tile_pool(name="psS", bufs=1, space="PSUM")
        psO = tc.alloc_tile_pool(name="psO", bufs=1, space="PSUM")

        oT_sb = ot_p.tile([128, PC, N], BF16, tag="oT")
        scale = float(HD) ** -0.5

        units = [(ih, hp) for ih in range(2) for hp in range(H // 2)]
        staged = {}
        ps_big = psS.tile([128, 4, 512], F32, tag="sbig", name="ps_big")
        slot_ctr = [0]

        def stage1(u):
            ih, hp = units[u]
            i0 = ih * 512
            pts = [
                pt_p.tile([128, PC, 512], BF16, tag=f"pt{sub}", name="pt_t")
                for sub in range(2)
            ]
            for j0 in range(0, PC, 2):
                slots = []
                for sub in range(2):
                    s = slot_ctr[0] % 2
                    slot_ctr[0] += 1
                    slots.append(ps_big[:, 2 * s : 2 * s + 2, :])
                for dj in range(2):
                    for sub in range(2):  # adjacent row-group pair: concurrent
                        p0 = sub * 64
                        nc.tensor.matmul(
                            slots[sub][:, dj, :],
                            kT_sb[
                                p0 : p0 + 64,
                                hp,
                                (j0 + dj) * 128 : (j0 + dj + 1) * 128,
                            ],
                            qT_sb[p0 : p0 + 64, hp, i0 : i0 + 512],
                        )
                for sub in range(2):
                    nc.scalar.activation(
                        pts[sub][:, j0 : j0 + 2, :].rearrange("p j i -> p (j i)"),
                        slots[sub].rearrange("p j i -> p (j i)"),
                        EXP,
                        scale=scale,
                    )
            staged[u] = pts

        def stage2(u):
            ih, hp = units[u]
            i0 = ih * 512
            pts = staged.pop(u)
            for sub in range(2):
                h = 2 * hp + sub
                p0 = sub * 64
                pt = pts[sub]
                ps_o = psO.tile([HD + 1, 512], F32, tag=f"o{sub}", name="psO_t")
                for j in range(PC):
                    nc.tensor.matmul(
                        ps_o[:],
                        vp_sb[:, j, h, :],
                        pt[:, j, :],
                        start=(j == 0),
                        stop=(j == PC - 1),
                    )
                zraw = rz_p.tile([1, 512], F32, tag="rz", name="rz_t")
                nc.vector.tensor_copy(zraw[:], ps_o[HD : HD + 1, :])
                zbc = zb_p.tile([HD, 512], F32, tag="zbc", name="zbc_t")
                nc.gpsimd.partition_broadcast(zbc[:], zraw[:], channels=HD)
                zb = zb_p.tile([HD, 512], F32, tag="zb", name="zb_t")
                nc.vector.reciprocal_approx_fast(zb[:], zbc[:])
                otmp = zb_p.tile([HD, 512], F32, tag="otmp", name="otmp_t")
                nc.vector.tensor_mul(otmp[:], ps_o[0:HD, :], zb[:])
                nc.vector.tensor_copy(
                    oT_sb[p0 : p0 + 64, hp, i0 : i0 + 512], otmp[:]
                )

        def proj_group(fo, th):
            wps = wps_p.tile([128, PC, 128], BF16, tag="wps", name="wps")
            nc.sync.dma_start(
                out=wps[:],
                in_=wprojT_d[:, fo * 128 : (fo + 1) * 128].rearrange(
                    "(cc p) f -> p cc f", p=128
                ),
            )
            ps = psB.tile([128, 512], F32, tag="psB", name="psB_t")
            for cc in range(PC):
                nc.tensor.matmul(
                    ps[:],
                    wps[:, cc, :],
                    oT_sb[:, cc, th * 512 : (th + 1) * 512],
                    start=(cc == 0),
                    stop=(cc == PC - 1),
                )
            y_sb = y_p.tile([128, 512], F32, tag="y", name="y_t")
            nc.vector.tensor_scalar_add(y_sb[:], ps[:], bias_sb[:, fo, :])
            nc.sync.dma_start(
                out=yT_d[fo * 128 : (fo + 1) * 128, th * 512 : (th + 1) * 512],
                in_=y_sb[:],
            )

        # prologue: two stage1 units, then the V' build (full-array PE work
        # that overlaps the first exps), then the steady-state pipeline.
        stage1(0)
        stage1(1)

        vslab_p = tc.alloc_tile_pool(name="vslab", bufs=1, side="right")
        for fh in range(2):
            vslab = col_slab(wqkvT_d, vslab_p, "vslab", 2 * C + fh * 512, 512)
            for tc_i in range(PC):
                ps = psB.tile([128, 512], F32, tag="psB", name="psB_t")
                for cc in range(PC):
                    nc.tensor.matmul(
                        ps[:],
                        x_sb[:, cc, tc_i * 128 : (tc_i + 1) * 128],
                        vslab[:, cc, :],
                        start=(cc == 0),
                        stop=False,
                    )
                for cc in range(PC):
                    nc.tensor.matmul(
                        ps[:],
                        x_sb[:, cc, tc_i * 128 : (tc_i + 1) * 128],
                        wv_sb[:, cc, fh * 512 : (fh + 1) * 512],
                        start=False,
                        stop=(cc == PC - 1),
                    )
                nc.scalar.copy(
                    vp_sb[:, tc_i, fh * 8 : (fh + 1) * 8, 0:HD],
                    ps[:].rearrange("p (h d) -> p h d", d=HD),
                )
        for tc_i in range(PC):
            nc.vector.memset(vp_sb[:, tc_i, :, HD : HD + 1], 1.0)
        vslab_p.release()

        stage2(0)
        # steady state: stage1 one unit ahead of stage2; the first
        # output-projection half interleaves once its oT half is complete.
        for i in range(2, len(units) + 1):
            if i < len(units):
                stage1(i)
            stage2(i - 1)
            done = i - 1  # units[done] just finished stage2
            if 7 <= done <= 14:
                proj_group(done - 7, 0)

        x_p.release()
        wv_p.release()

        zb_p.release()
        rz_p.release()
        pt_p.release()
        psO.release()
        psS.release()

        # ---- remaining projection half ----
        for fo in range(PC):
            proj_group(fo, 1)

        y_p.release()
        wps_p.release()
        ot_p.release()
        vp_p.release()
        qt_p.release()
        kt_p.release()
        psB.release()
        small_p.release()

    nc.compile()
    return nc


_CACHE = {}


def _get_nc():
    if "nc" not in _CACHE:
        _CACHE["nc"] = _build()
    return _CACHE["nc"]


def _host_prep(x, W_qkv, W_proj, b_proj, coef_k, coef_v, indices, task):
    x = np.asarray(x, dtype=np.float32)
    W_qkv = np.asarray(W_qkv, dtype=np.float32)
    W_proj = np.asarray(W_proj, dtype=np.float32)
    b_proj = np.asarray(b_proj, dtype=np.float32)
    coef_k = np.asarray(coef_k, dtype=np.float32)
    coef_v = np.asarray(coef_v, dtype=np.float32)
    indices = np.asarray(indices)
    t = int(np.asarray(task).reshape(())) + 1

    assert x.shape == (B, N, C), x.shape

    # Host-side input marshaling: scatter the per-task frequency coefficients
    # into dense C x C planes (the sum across tasks commutes with the linear
    # inverse DCT), exactly as the reference does before its matmuls.
    def scatter(coef, idx):
        s = np.zeros(C * C, dtype=np.float32)
        np.add.at(s, idx.reshape(-1).astype(np.int64), coef.reshape(-1))
        return s.reshape(C, C)

    bm = _dct_matrix(C)
    sk = scatter(coef_k[:t], indices[:t])
    sv = scatter(coef_v[:t], indices[:t])
    import ml_dtypes

    shared = {
        "wqkvT": np.ascontiguousarray(W_qkv.T),
        "wprojT": np.ascontiguousarray(W_proj.T).astype(ml_dtypes.bfloat16),
        "bias": np.ascontiguousarray(b_proj.reshape(C, 1)),
        "bm": bm,
    }
    maps = []
    for b in range(NCORES):
        fq = b % 4
        maps.append(
            {
                "xT": np.ascontiguousarray(x[b].T),
                "sw": sk if b < 4 else sv,
                "bmq": np.ascontiguousarray(bm[:, fq * 256 : (fq + 1) * 256]),
                **shared,
            }
        )
    return maps


def kernel(x, W_qkv, W_proj, b_proj, coef_k, coef_v, indices, task):
    in_maps = _host_prep(x, W_qkv, W_proj, b_proj, coef_k, coef_v, indices, task)
    nc = _get_nc()
    res = run_bass_kernel_spmd(nc, in_maps, list(range(NCORES)))

    out = np.empty((B, N, C), dtype=np.float32)
    for b in range(NCORES):
        out[b] = res.results[b]["yT"].T
    return out



# revision 7
# speedup vs baseline: 1.4365x; 1.4365x over previous
"""Trainium2 Bass kernel for nn_Attention_LoRA_FFT.

Sharding: data-parallel over batch B=8 across the 8 NeuronCores. The DCT
LoRA weight reconstruction is sharded: each core builds a 256-column
slice of one of WkT/WvT and an AllGather distributes the full weights.

v2 structure (vs v1): everything bf16; all gather-independent matmuls
(qT, k_qkv, v_qkv) run before the gather result is needed, the LoRA
contribution is applied afterwards as accumulate-in-place passes; the
attention pipeline is gated per head-pair parity on the gather half it
needs and statically interleaved with the LoRA/projection matmuls; the
exp PSUM slots are double-buffered so the PE never waits on ACT.
"""

import os
import sys

for _p in ("/opt/trn_rl_repo", "/root/.axon_site/_ro/trn_rl_repo"):
    if os.path.isdir(_p) and _p not in sys.path:
        sys.path.insert(0, _p)

import numpy as np

import concourse.bacc as bacc
import concourse.mybir as mybir
from concourse.tile import TileContext
from concourse.bass_utils import run_bass_kernel_spmd

B, N, C = 8, 1024, 1024
H, HD = 16, 64
NCORES = 8
PC = C // 128
F32 = mybir.dt.float32
BF16 = mybir.dt.bfloat16
EXP = mybir.ActivationFunctionType.Exp


def _dct_matrix(n: int) -> np.ndarray:
    i = np.arange(n, dtype=np.float32)[:, None]
    j = np.arange(n, dtype=np.float32)[None, :]
    m = np.sqrt(np.float32(2.0 / n)) * np.cos(
        np.float32(np.pi) * i * (2.0 * j + 1.0) / np.float32(2.0 * n)
    )
    m[0, :] = np.sqrt(np.float32(1.0 / n))
    return m.astype(np.float32)


def _build():
    nc = bacc.Bacc("TRN2", target_bir_lowering=False, debug=False, num_devices=NCORES)

    xT_d = nc.dram_tensor("xT", [C, N], BF16, kind="ExternalInput")
    wqkvT_d = nc.dram_tensor("wqkvT", [C, 3 * C], BF16, kind="ExternalInput")
    wprojT_d = nc.dram_tensor("wprojT", [C, C], BF16, kind="ExternalInput")
    bias_d = nc.dram_tensor("bias", [C, 1], F32, kind="ExternalInput")
    bm_d = nc.dram_tensor("bm", [C, C], BF16, kind="ExternalInput")
    sw_d = nc.dram_tensor("sw", [C, C], BF16, kind="ExternalInput")
    bmq_d = nc.dram_tensor("bmq", [C, 256], BF16, kind="ExternalInput")
    yT_d = nc.dram_tensor("yT", [C, N], F32, kind="ExternalOutput")
    cc_in0 = nc.dram_tensor("cc_in0", [C, 128], BF16)
    cc_in1 = nc.dram_tensor("cc_in1", [C, 128], BF16)
    cc_out0 = nc.dram_tensor("cc_out0", [NCORES * C, 128], BF16, addr_space="Shared")
    cc_out1 = nc.dram_tensor("cc_out1", [NCORES * C, 128], BF16, addr_space="Shared")

    def col_slab(q, dram_ap, pool, tag, f0, width):
        slab = pool.tile([128, PC, width], BF16, tag=tag, name=tag)
        q.dma_start(
            out=slab[:],
            in_=dram_ap[:, f0 : f0 + width].rearrange("(cc p) f -> p cc f", p=128),
        )
        return slab

    with TileContext(nc) as tc:
        # ---------------- left stack ----------------
        small_p = tc.alloc_tile_pool(name="small", bufs=1, side="left")
        bias_sb = small_p.tile([128, PC, 1], F32, tag="bias")
        nc.scalar.dma_start(
            out=bias_sb[:], in_=bias_d.rearrange("(cc p) o -> p cc o", p=128)
        )

        wv_p = tc.alloc_tile_pool(name="wvp", bufs=1, side="left")
        x_p = tc.alloc_tile_pool(name="xp", bufs=1, side="left")
        wk_p = tc.alloc_tile_pool(name="wkp", bufs=1, side="left")
        wv_sb = wv_p.tile([128, PC, C], BF16, tag="wv")
        wk_sb = wk_p.tile([128, PC, C], BF16, tag="wk")
        x_sb = x_p.tile([128, PC, N], BF16, tag="x")

        # ================= Phase A: sharded LoRA reconstruction =======
        slabA_p = tc.alloc_tile_pool(name="slabA", bufs=3, side="right")
        bm_p = tc.alloc_tile_pool(name="bmp", bufs=1, side="right")
        bmq_p = tc.alloc_tile_pool(name="bmqp", bufs=1, side="right")
        g_p = tc.alloc_tile_pool(name="gp", bufs=1, side="right")
        wpart_p = tc.alloc_tile_pool(name="wpartp", bufs=1, side="right")
        psA = tc.alloc_tile_pool(name="psA", bufs=4, space="PSUM")

        bmq_sb = bmq_p.tile([128, PC, 256], BF16, tag="bmq")
        nc.scalar.dma_start(
            out=bmq_sb[:], in_=bmq_d.rearrange("(cc p) f -> p cc f", p=128)
        )
        bm_sb = bm_p.tile([128, PC, C], BF16, tag="bm")
        nc.scalar.dma_start(
            out=bm_sb[:], in_=bm_d.rearrange("(cc p) f -> p cc f", p=128)
        )
        # x early on the scalar queue (needed right after the recon)
        for cc in range(PC):
            nc.scalar.dma_start(
                out=x_sb[:, cc, :], in_=xT_d[cc * 128 : (cc + 1) * 128, :]
            )

        g_sb = g_p.tile([128, PC, 256], BF16, tag="g", name="g_sb")
        wpart_sb = wpart_p.tile([128, PC, 256], BF16, tag="wpart", name="wpart_sb")
        for at in range(PC):
            slab = col_slab(nc.sync, sw_d, slabA_p, "slabA", at * 128, 128)
            ps = psA.tile([128, 256], F32, tag="psA", name="psA_t")
            for bc in range(PC):
                nc.tensor.matmul(
                    ps[:],
                    slab[:, bc, :],
                    bmq_sb[:, bc, :],
                    start=(bc == 0),
                    stop=(bc == PC - 1),
                )
            nc.scalar.copy(g_sb[:, at, :], ps[:])
        # second chain per 128-col half so the first gather can launch early
        for hf, cc_in in ((0, cc_in0), (1, cc_in1)):
            for ct in range(PC):
                ps = psA.tile([128, 128], F32, tag="psA2", name="psA2_t")
                for ac in range(PC):
                    nc.tensor.matmul(
                        ps[:],
                        bm_sb[:, ac, ct * 128 : (ct + 1) * 128],
                        g_sb[:, ac, hf * 128 : (hf + 1) * 128],
                        start=(ac == 0),
                        stop=(ac == PC - 1),
                    )
                nc.scalar.copy(wpart_sb[:, ct, hf * 128 : (hf + 1) * 128], ps[:])
            nc.scalar.dma_start(
                out=cc_in.rearrange("(ct p) f -> p ct f", p=128),
                in_=wpart_sb[:, :, hf * 128 : (hf + 1) * 128],
            )

        # ---- trigger both gathers; read back on the gpsimd queue ----
        for hf, cc_in, cc_out in ((0, cc_in0, cc_out0), (1, cc_in1, cc_out1)):
            nc.gpsimd.collective_compute(
                "AllGather",
                mybir.AluOpType.bypass,
                replica_groups=[list(range(NCORES))],
                ins=[cc_in[:]],
                outs=[cc_out[:]],
            )
            # order: wk fq0, wk fq1, wv fq0..3, wk fq2, wk fq3 is overkill;
            # simple order wk0,wk1,wv0,wv1,wk2,wv2,wk3,wv3 keeps both the
            # k-lora and v-lora start latency low.
            order = [(0, 0), (0, 1), (1, 0), (1, 1), (0, 2), (1, 2), (0, 3), (1, 3)]
            for wi, fq in order:
                w_sb = wk_sb if wi == 0 else wv_sb
                base = (wi * 4 + fq) * C
                nc.gpsimd.dma_start(
                    out=w_sb[:, :, fq * 256 + hf * 128 : fq * 256 + (hf + 1) * 128],
                    in_=cc_out[base : base + C, :].rearrange(
                        "(cc p) f -> p cc f", p=128
                    ),
                )

        psA.release()
        wpart_p.release()
        g_p.release()
        bmq_p.release()
        bm_p.release()
        slabA_p.release()

        # ================= Phase B: gather-independent matmuls =========
        kt_p = tc.alloc_tile_pool(name="ktp", bufs=1, side="right")
        qt_p = tc.alloc_tile_pool(name="qtp", bufs=1, side="right")
        vp_p = tc.alloc_tile_pool(name="vpp", bufs=1, side="right")
        slabB_p = tc.alloc_tile_pool(name="slabB", bufs=4, side="right")
        psB = tc.alloc_tile_pool(name="psB", bufs=2, space="PSUM", side="right")

        kT_sb = kt_p.tile([128, PC, N], BF16, tag="kT")
        qT_sb = qt_p.tile([128, PC, N], BF16, tag="qT")
        vp_sb = vp_p.tile([128, PC, H, HD + 1], BF16, tag="vp")

        # ---- qT ----
        for fc in range(PC):
            slab = col_slab(nc.sync, wqkvT_d, slabB_p, "slabB", fc * 128, 128)
            for th in range(2):
                ps = psB.tile([128, 512], F32, tag="psB", name="psB_t")
                for cc in range(PC):
                    nc.tensor.matmul(
                        ps[:],
                        slab[:, cc, :],
                        x_sb[:, cc, th * 512 : (th + 1) * 512],
                        start=(cc == 0),
                        stop=(cc == PC - 1),
                    )
                nc.scalar.copy(qT_sb[:, fc, th * 512 : (th + 1) * 512], ps[:])

        # ---- k_qkv into kT_sb (lora added in place later) ----
        for fc in range(PC):
            slab = col_slab(nc.sync, wqkvT_d, slabB_p, "slabB", C + fc * 128, 128)
            for th in range(2):
                ps = psB.tile([128, 512], F32, tag="psB", name="psB_t")
                for cc in range(PC):
                    nc.tensor.matmul(
                        ps[:],
                        slab[:, cc, :],
                        x_sb[:, cc, th * 512 : (th + 1) * 512],
                        start=(cc == 0),
                        stop=(cc == PC - 1),
                    )
                nc.scalar.copy(kT_sb[:, fc, th * 512 : (th + 1) * 512], ps[:])

        # ---- v_qkv into vp_sb ----
        vslab_p = tc.alloc_tile_pool(name="vslab", bufs=2, side="right")
        for fh in range(2):
            vslab = col_slab(nc.sync, wqkvT_d, vslab_p, "vslab", 2 * C + fh * 512, 512)
            for tc_i in range(PC):
                ps = psB.tile([128, 512], F32, tag="psB", name="psB_t")
                for cc in range(PC):
                    nc.tensor.matmul(
                        ps[:],
                        x_sb[:, cc, tc_i * 128 : (tc_i + 1) * 128],
                        vslab[:, cc, :],
                        start=(cc == 0),
                        stop=(cc == PC - 1),
                    )
                nc.scalar.copy(
                    vp_sb[:, tc_i, fh * 8 : (fh + 1) * 8, 0:HD],
                    ps[:].rearrange("p (h d) -> p h d", d=HD),
                )
        for tc_i in range(PC):
            nc.vector.memset(vp_sb[:, tc_i, :, HD : HD + 1], 1.0)
        vslab_p.release()
        slabB_p.release()

        # ================= Phase C: lora apply + attention + proj ======
        ot_p = tc.alloc_tile_pool(name="otp", bufs=1, side="right")
        wps_p = tc.alloc_tile_pool(name="wpsp", bufs=2, side="right")
        y_p = tc.alloc_tile_pool(name="yp", bufs=2, side="right")
        pt_p = tc.alloc_tile_pool(name="ptp", bufs=2, side="right")
        rz_p = tc.alloc_tile_pool(name="rzp", bufs=2, side="right")
        zb_p = tc.alloc_tile_pool(name="zbp", bufs=2, side="right")
        psS = tc.alloc_tile_pool(name="psS", bufs=2, space="PSUM")
        psO = tc.alloc_tile_pool(name="psO", bufs=2, space="PSUM")

        oT_sb = ot_p.tile([128, PC, N], BF16, tag="oT")
        scale = float(HD) ** -0.5

        def k_lora(fc):
            # kT[:, fc, :] += (Wk @ x.T)[fc block]
            for th in range(2):
                ps = psB.tile([128, 512], F32, tag="psB", name="psB_t")
                for cc in range(PC):
                    nc.tensor.matmul(
                        ps[:],
                        wk_sb[:, cc, fc * 128 : (fc + 1) * 128],
                        x_sb[:, cc, th * 512 : (th + 1) * 512],
                        start=(cc == 0),
                        stop=(cc == PC - 1),
                    )
                dst = kT_sb[:, fc, th * 512 : (th + 1) * 512]
                nc.vector.tensor_add(dst, dst, ps[:])

        def v_lora(hf, tcs):
            # vp[:, tc, heads of parity hf, :HD] += (x @ Wv.T) strips
            for tc_i in tcs:
                ps = psB.tile([128, 512], F32, tag="psB", name="psB_t")
                for fq in range(4):
                    f0 = fq * 256 + hf * 128
                    for cc in range(PC):
                        nc.tensor.matmul(
                            ps[:, fq * 128 : (fq + 1) * 128],
                            x_sb[:, cc, tc_i * 128 : (tc_i + 1) * 128],
                            wv_sb[:, cc, f0 : f0 + 128],
                            start=(cc == 0),
                            stop=(cc == PC - 1),
                        )
                # head h = 4*fq + 2*hf + s owns feature cols fq*256+hf*128+s*64
                src = ps[:].rearrange("p (fq s d) -> p fq s d", fq=4, s=2)
                dst = vp_sb[:, tc_i, :, 0:HD].rearrange(
                    "p (fq g s) d -> p g fq s d", fq=4, g=2, s=2
                )[:, hf]
                nc.vector.tensor_add(dst, dst, src)

        units = (
            [(0, hp) for hp in (0, 2, 4, 6)]
            + [(0, hp) for hp in (1, 3, 5, 7)]
            + [(1, hp) for hp in (0, 2, 4, 6)]
            + [(1, hp) for hp in (1, 3, 5, 7)]
        )
        staged = {}

        def stage1(u):
            ih, hp = units[u]
            i0 = ih * 512
            pt = pt_p.tile([128, 2, PC, 512], BF16, tag="pt", name="pt_t")
            for j in range(PC):
                ps = psS.tile([128, 2, 512], F32, tag="psS", name="psS_t")
                for sub in range(2):
                    p0 = sub * 64
                    nc.tensor.matmul(
                        ps[:, sub, :],
                        kT_sb[p0 : p0 + 64, hp, j * 128 : (j + 1) * 128],
                        qT_sb[p0 : p0 + 64, hp, i0 : i0 + 512],
                    )
                nc.scalar.activation(pt[:, :, j, :], ps[:], EXP, scale=scale)
            staged[u] = pt

        def stage2(u):
            ih, hp = units[u]
            i0 = ih * 512
            pt = staged.pop(u)
            for sub in range(2):
                h = 2 * hp + sub
                p0 = sub * 64
                ps_o = psO.tile([HD + 1, 512], F32, tag="psO", name="psO_t")
                for j in range(PC):
                    nc.tensor.matmul(
                        ps_o[:],
                        vp_sb[:, j, h, :],
                        pt[:, sub, j, :],
                        start=(j == 0),
                        stop=(j == PC - 1),
                    )
                zraw = rz_p.tile([1, 512], F32, tag="rz", name="rz_t")
                nc.vector.tensor_copy(zraw[:], ps_o[HD : HD + 1, :])
                rz = rz_p.tile([1, 512], F32, tag="rzr", name="rzr_t")
                nc.vector.reciprocal_approx_fast(rz[:], zraw[:])
                zb = zb_p.tile([HD, 512], F32, tag="zb", name="zb_t")
                nc.gpsimd.partition_broadcast(zb[:], rz[:], channels=HD)
                nc.vector.tensor_mul(
                    oT_sb[p0 : p0 + 64, hp, i0 : i0 + 512], ps_o[0:HD, :], zb[:]
                )

        def proj_group(fo, th):
            wps = wps_p.tile([128, PC, 128], BF16, tag="wps", name="wps")
            nc.sync.dma_start(
                out=wps[:],
                in_=wprojT_d[:, fo * 128 : (fo + 1) * 128].rearrange(
                    "(cc p) f -> p cc f", p=128
                ),
            )
            ps = psB.tile([128, 512], F32, tag="psB", name="psB_t")
            for cc in range(PC):
                nc.tensor.matmul(
                    ps[:],
                    wps[:, cc, :],
                    oT_sb[:, cc, th * 512 : (th + 1) * 512],
                    start=(cc == 0),
                    stop=(cc == PC - 1),
                )
            y_sb = y_p.tile([128, 512], F32, tag="y", name="y_t")
            nc.vector.tensor_scalar_add(y_sb[:], ps[:], bias_sb[:, fo, :])
            nc.sync.dma_start(
                out=yT_d[fo * 128 : (fo + 1) * 128, th * 512 : (th + 1) * 512],
                in_=y_sb[:],
            )

        # ---- static interleave schedule ----
        # NB Tile orders strictly by program order: every stage1(u) must come
        # after the k_lora of its hp, every stage2(u) after all v_lora of its
        # head parity.
        k_lora(0)
        stage1(0)
        k_lora(2)
        stage1(1)
        v_lora(0, [0, 1, 2, 3])
        v_lora(0, [4, 5, 6, 7])
        stage2(0)
        k_lora(4)
        stage1(2)
        stage2(1)
        k_lora(6)
        stage1(3)
        stage2(2)
        # gather1-dependent section
        k_lora(1)
        stage1(4)
        stage2(3)
        v_lora(1, [0, 1, 2, 3])
        k_lora(3)
        stage1(5)
        v_lora(1, [4, 5, 6, 7])
        stage2(4)
        k_lora(5)
        stage1(6)
        stage2(5)
        k_lora(7)
        stage1(7)
        stage2(6)
        stage1(8)
        stage2(7)
        for i in range(9, len(units) + 1):
            if i < len(units):
                stage1(i)
            stage2(i - 1)
            done = i - 1
            if 8 <= done <= 15:
                proj_group(done - 8, 0)

        wk_p.release()
        x_p.release()
        wv_p.release()

        zb_p.release()
        rz_p.release()
        pt_p.release()
        psO.release()
        psS.release()

        # ---- remaining projection half ----
        for fo in range(PC):
            proj_group(fo, 1)

        y_p.release()
        wps_p.release()
        ot_p.release()
        vp_p.release()
        qt_p.release()
        kt_p.release()
        psB.release()
        small_p.release()

    nc.compile()
    return nc


_CACHE = {}


def _get_nc():
    if "nc" not in _CACHE:
        _CACHE["nc"] = _build()
    return _CACHE["nc"]


def _host_prep(x, W_qkv, W_proj, b_proj, coef_k, coef_v, indices, task):
    import ml_dtypes

    bf16 = ml_dtypes.bfloat16
    x = np.asarray(x, dtype=np.float32)
    W_qkv = np.asarray(W_qkv, dtype=np.float32)
    W_proj = np.asarray(W_proj, dtype=np.float32)
    b_proj = np.asarray(b_proj, dtype=np.float32)
    coef_k = np.asarray(coef_k, dtype=np.float32)
    coef_v = np.asarray(coef_v, dtype=np.float32)
    indices = np.asarray(indices)
    t = int(np.asarray(task).reshape(())) + 1

    assert x.shape == (B, N, C), x.shape

    # Host-side input marshaling: scatter the per-task frequency coefficients
    # into dense C x C planes (the sum across tasks commutes with the linear
    # inverse DCT), exactly as the reference does before its matmuls.
    def scatter(coef, idx):
        s = np.zeros(C * C, dtype=np.float32)
        np.add.at(s, idx.reshape(-1).astype(np.int64), coef.reshape(-1))
        return s.reshape(C, C)

    bm = _dct_matrix(C)
    sk = scatter(coef_k[:t], indices[:t])
    sv = scatter(coef_v[:t], indices[:t])

    shared = {
        "wqkvT": np.ascontiguousarray(W_qkv.T).astype(bf16),
        "wprojT": np.ascontiguousarray(W_proj.T).astype(bf16),
        "bias": np.ascontiguousarray(b_proj.reshape(C, 1)),
        "bm": bm.astype(bf16),
    }
    maps = []
    for b in range(NCORES):
        fq = b % 4
        maps.append(
            {
                "xT": np.ascontiguousarray(x[b].T).astype(bf16),
                "sw": (sk if b < 4 else sv).astype(bf16),
                "bmq": np.ascontiguousarray(
                    bm[:, fq * 256 : (fq + 1) * 256]
                ).astype(bf16),
                **shared,
            }
        )
    return maps


def kernel(x, W_qkv, W_proj, b_proj, coef_k, coef_v, indices, task):
    in_maps = _host_prep(x, W_qkv, W_proj, b_proj, coef_k, coef_v, indices, task)
    nc = _get_nc()
    res = run_bass_kernel_spmd(nc, in_maps, list(range(NCORES)))

    out = np.empty((B, N, C), dtype=np.float32)
    for b in range(NCORES):
        out[b] = res.results[b]["yT"].T
    return out
